# revision 2
# baseline (speedup 1.0000x reference)
"""Distributed forward pass for nn_AGC_85126251807219 (gnn_message_passing).

Strategy (per sharding hint): data-parallel over bs across the 8
NeuronCores; the global softmax over all E = bs*n edge scalars uses a
cross-device max/sum all-reduce, and the training-mode BatchNorm batch
stats use cross-device mean/var all-reduces (psum of per-channel sum and
sum-of-squares). Weights are replicated.

kernel(**inputs) takes FULL unsharded inputs and returns the FULL output.
"""

import numpy as np

EPS = 1e-5
SLOPE = 0.01

BS, N, F = 512, 676, 128
NCORES = 8
E_TOT = float(BS * N)


def _forward_shard(axis_name, x, w_init, W1, b1, g1, be1, W2, b2, g2, be2,
                   W3, b3, g3, be3, W4, b4, g4, be4, Wl, bl,
                   Wfc, bfc, gfc, befc):
    import jax
    import jax.numpy as jnp

    bs_l, n, f = x.shape
    hub = x[:, :1, :]                          # [bs_l,1,f]
    diff = jnp.abs(hub - x).reshape(-1, f)     # [E_l,f]

    def bn(z, g, b):
        s1 = jax.lax.psum(jnp.sum(z, axis=0), axis_name)
        s2 = jax.lax.psum(jnp.sum(z * z, axis=0), axis_name)
        m = s1 / E_TOT
        v = s2 / E_TOT - m * m
        return (z - m) * jax.lax.rsqrt(v + EPS) * g + b

    h = diff
    for W, b, g, be in ((W1, b1, g1, be1), (W2, b2, g2, be2),
                        (W3, b3, g3, be3), (W4, b4, g4, be4)):
        h = jax.nn.leaky_relu(bn(h @ W.T + b, g, be), SLOPE)

    w_raw = h @ Wl.T + bl                      # [E_l,1]
    w0 = w_init.reshape(-1, 1)                 # [E_l,1]
    d = (w_raw - w0).reshape(-1)
    gmax = jax.lax.pmax(jnp.max(d), axis_name)
    u = jnp.exp(d - gmax)
    gsum = jax.lax.psum(jnp.sum(u), axis_name)
    w1 = (u / gsum).reshape(-1, 1)             # [E_l,1]

    H = jnp.broadcast_to(hub, (bs_l, n, f)).reshape(-1, f)
    m_cat = jnp.concatenate([w0 * H, w1 * H], axis=1)   # [E_l,2f]
    out = bn(m_cat @ Wfc.T + bfc, gfc, befc)
    return out.reshape(bs_l, n, -1)


def _run_pmap(inputs, devices):
    import jax
    from functools import partial

    names = ["x", "w_init", "W1", "b1", "g1", "be1", "W2", "b2", "g2", "be2",
             "W3", "b3", "g3", "be3", "W4", "b4", "g4", "be4", "Wl", "bl",
             "Wfc", "bfc", "gfc", "befc"]
    args = [np.asarray(inputs[k]) for k in names]
    in_axes = tuple([0, 0] + [None] * (len(names) - 2))

    fn = jax.pmap(partial(_forward_shard, "i"), axis_name="i",
                  in_axes=in_axes, devices=devices)

    # shard x / w_init over bs
    args[0] = args[0].reshape(NCORES, BS // NCORES, N, F)
    args[1] = args[1].reshape(NCORES, BS // NCORES, N, 1)
    out = fn(*args)
    return np.asarray(out).reshape(BS, N, -1).astype(np.float32)


def _run_numpy(inputs):
    x = inputs["x"].astype(np.float64)
    hub = x[:, :1, :]
    diff = np.abs(hub - x).reshape(-1, F)

    def bn(z, g, b):
        m = z.mean(axis=0)
        v = z.var(axis=0)
        return (z - m) / np.sqrt(v + EPS) * g + b

    def lrelu(z):
        return np.where(z >= 0, z, SLOPE * z)

    h = diff
    for W, b, g, be in (("W1", "b1", "g1", "be1"), ("W2", "b2", "g2", "be2"),
                        ("W3", "b3", "g3", "be3"), ("W4", "b4", "g4", "be4")):
        h = lrelu(bn(h @ inputs[W].T.astype(np.float64) + inputs[b], inputs[g], inputs[be]))
    w_raw = h @ inputs["Wl"].T.astype(np.float64) + inputs["bl"]
    w0 = inputs["w_init"].reshape(-1, 1).astype(np.float64)
    d = (w_raw - w0).reshape(-1)
    u = np.exp(d - d.max())
    w1 = (u / u.sum()).reshape(-1, 1)
    H = np.broadcast_to(hub, x.shape).reshape(-1, F)
    m_cat = np.concatenate([w0 * H, w1 * H], axis=1)
    out = bn(m_cat @ inputs["Wfc"].T.astype(np.float64) + inputs["bfc"],
             inputs["gfc"], inputs["befc"])
    return out.reshape(BS, N, -1).astype(np.float32)


def kernel(**inputs):
    try:
        import jax
        devs = [d for d in jax.devices() if d.platform != "cpu"][:NCORES]
        if len(devs) == NCORES:
            return _run_pmap(inputs, devs)
    except Exception:
        pass
    return _run_numpy(inputs)


# revision 3
# speedup vs baseline: 3.0243x; 3.0243x over previous
"""Distributed forward pass for nn_AGC_85126251807219 (gnn_message_passing).

Strategy (per sharding hint): data-parallel over bs across the 8
NeuronCores; the global softmax over all E = bs*n edge scalars uses a
cross-device max/sum all-reduce, and the training-mode BatchNorm batch
stats use cross-device mean/var all-reduces (psum of per-channel sum and
sum-of-squares). Weights are replicated.

kernel(**inputs) takes FULL unsharded inputs and returns the FULL output.
"""

import numpy as np

EPS = 1e-5
SLOPE = 0.01

BS, N, F = 512, 676, 128
NCORES = 8
E_TOT = float(BS * N)


def _forward_shard(axis_name, x, w_init, W1, b1, g1, be1, W2, b2, g2, be2,
                   W3, b3, g3, be3, W4, b4, g4, be4, Wl, bl,
                   Wfc, bfc, gfc, befc):
    import jax
    import jax.numpy as jnp

    bs_l, n, f = x.shape
    hub = x[:, :1, :]                          # [bs_l,1,f]
    diff = jnp.abs(hub - x).reshape(-1, f)     # [E_l,f]

    def bn(z, g, b):
        s1 = jax.lax.psum(jnp.sum(z, axis=0), axis_name)
        s2 = jax.lax.psum(jnp.sum(z * z, axis=0), axis_name)
        m = s1 / E_TOT
        v = s2 / E_TOT - m * m
        return (z - m) * jax.lax.rsqrt(v + EPS) * g + b

    h = diff
    for W, b, g, be in ((W1, b1, g1, be1), (W2, b2, g2, be2),
                        (W3, b3, g3, be3), (W4, b4, g4, be4)):
        h = jax.nn.leaky_relu(bn(h @ W.T + b, g, be), SLOPE)

    w_raw = h @ Wl.T + bl                      # [E_l,1]
    w0 = w_init.reshape(-1, 1)                 # [E_l,1]
    d = (w_raw - w0).reshape(-1)
    gmax = jax.lax.pmax(jnp.max(d), axis_name)
    u = jnp.exp(d - gmax)
    gsum = jax.lax.psum(jnp.sum(u), axis_name)
    w1 = (u / gsum).reshape(-1, 1)             # [E_l,1]

    H = jnp.broadcast_to(hub, (bs_l, n, f)).reshape(-1, f)
    m_cat = jnp.concatenate([w0 * H, w1 * H], axis=1)   # [E_l,2f]
    out = bn(m_cat @ Wfc.T + bfc, gfc, befc)
    return out.reshape(bs_l, n, -1)


_PMAP_CACHE = {}


def _run_pmap(inputs, devices):
    import jax
    from functools import partial

    names = ["x", "w_init", "W1", "b1", "g1", "be1", "W2", "b2", "g2", "be2",
             "W3", "b3", "g3", "be3", "W4", "b4", "g4", "be4", "Wl", "bl",
             "Wfc", "bfc", "gfc", "befc"]
    args = [np.asarray(inputs[k]) for k in names]
    in_axes = tuple([0, 0] + [None] * (len(names) - 2))

    key = tuple(id(d) for d in devices)
    fn = _PMAP_CACHE.get(key)
    if fn is None:
        fn = jax.pmap(partial(_forward_shard, "i"), axis_name="i",
                      in_axes=in_axes, devices=devices)
        _PMAP_CACHE[key] = fn

    # shard x / w_init over bs
    args[0] = args[0].reshape(NCORES, BS // NCORES, N, F)
    args[1] = args[1].reshape(NCORES, BS // NCORES, N, 1)
    out = fn(*args)
    return np.asarray(out).reshape(BS, N, -1).astype(np.float32)


def _run_numpy(inputs):
    x = inputs["x"].astype(np.float64)
    hub = x[:, :1, :]
    diff = np.abs(hub - x).reshape(-1, F)

    def bn(z, g, b):
        m = z.mean(axis=0)
        v = z.var(axis=0)
        return (z - m) / np.sqrt(v + EPS) * g + b

    def lrelu(z):
        return np.where(z >= 0, z, SLOPE * z)

    h = diff
    for W, b, g, be in (("W1", "b1", "g1", "be1"), ("W2", "b2", "g2", "be2"),
                        ("W3", "b3", "g3", "be3"), ("W4", "b4", "g4", "be4")):
        h = lrelu(bn(h @ inputs[W].T.astype(np.float64) + inputs[b], inputs[g], inputs[be]))
    w_raw = h @ inputs["Wl"].T.astype(np.float64) + inputs["bl"]
    w0 = inputs["w_init"].reshape(-1, 1).astype(np.float64)
    d = (w_raw - w0).reshape(-1)
    u = np.exp(d - d.max())
    w1 = (u / u.sum()).reshape(-1, 1)
    H = np.broadcast_to(hub, x.shape).reshape(-1, F)
    m_cat = np.concatenate([w0 * H, w1 * H], axis=1)
    out = bn(m_cat @ inputs["Wfc"].T.astype(np.float64) + inputs["bfc"],
             inputs["gfc"], inputs["befc"])
    return out.reshape(BS, N, -1).astype(np.float32)


def kernel(**inputs):
    try:
        import jax
        devs = [d for d in jax.devices() if d.platform != "cpu"][:NCORES]
        if len(devs) == NCORES:
            return _run_pmap(inputs, devs)
    except Exception:
        pass
    return _run_numpy(inputs)
